# revision 40
# baseline (speedup 1.0000x reference)
"""Trainium2 Bass kernel for nn_CandidateFinder (LSH hash-equality KNN).

Reference semantics: q/k binarized (x>0), projected by W [64,8], sign bits
packed into an 8-bit bucket code; for each query, return the first 64 key
indices (ascending) whose code equals the query's code, padded with -1.

Key insight: codes live in [0,256). Build, per batch, a per-bucket table of
the first 64 key indices, then gather per query. Both steps map onto
matmuls + a free-dim prefix scan + GPSIMD local_scatters.

Sharding: 8 cores = 4 batches x 2 bucket-halves (c in [0,128) / [128,256)).
Each core computes a partial gather (zero where the query's code is in the
other half); host sums the pair and subtracts 1 (table stores j+1, empty=0).

Pipeline per core (CoreSim 13539 ns vs the 14026 ns predecessor):
  - inputs host-prestacked to [128,512] tiles (chunk pairs on partition
    blocks 0:64/64:128) so every input DMA hits the 500ns dur floor and
    each binarize is a single [128,512] op.  Queues: SP kT0s+consts+qT0s,
    Pool kT1s+qT1s, ACT iota (a host constant, so Pool does not spend
    1.7us producing it; ACT-queue DMAs dispatch only after the act-table
    preload anyway).
  - hash lhsT compacted into the consts tensor as a [128,72] block
    diagonal per fp16 half (W = fp16 hi + fp16 lo accumulated in f32
    PSUM); bits land on psum rows 0:8/64:72.
  - k bits: pairs a,b as +-1 via ACT Sign; pair c as 0/1 via DVE is_gt.
    k one-hot {0,1} via ACT Relu (bias -7 for +-1 chunks, 1-pop(c) for
    0/1 chunks).
  - rank/compact: DVE runs the 5-piece chained prefix scan (256,256,512,
    512,512); the mask m1=onehot*rank runs on Pool for pieces 0-3 and on
    DVE for the last piece (no Pool round trip at the chain end); idx =
    m1-1 (DVE, int16) feeds 4 quarter local_scatters + 3 merges on Pool.
  - q side: bins on Pool, hashes on PE behind the k agrees, both sq
    halves as ACT Signs after the k relu chain; q one-hots: chunks 3,0 as
    ACT Relus, chunks 2,1 on DVE as is_gt(7) over bf16 psum views
    (strided odd columns pick the f32 high halves).
  - tail: 4 waves of 4 gather matmuls (512 queries each) in one-hot
    readiness order (3,2,0,1); copies w3/w0 on ACT, w2/w1 on DVE; DMAs
    w3,w1 on the SP queue and w0,w2 on the ACT queue.
add_dep hints pin the PE order (k agrees before the q hashes) and hold
the DVE q-one-hot tail behind the k sub chain; the list scheduler
otherwise inverts both and loses ~2us.

Measured critical path (CoreSim): kT0s lands 2417 (fixed DMA latency);
bins -> hash -> Sign -> agree -> Relu starts the scan at 4635; the ACT
relu cascade paces the scan/mask/scatter chain to the merged table at
~9.2us while ACT finishes sq + q one-hots at ~9.9us; the four
gather-copy-DMA waves issue by ~10.7us; the rest is fixed DMA completion
(~2.2us) and the final barrier (~0.6us).

Precision: the hash sign test needs ~f32-accurate projections. W is split
as fp16(W) + fp16(W - fp16(W)) and the two fp16 matmuls accumulate in f32
PSUM; representation error ~1e-6 vs hash sign margins ~1e-4 on this data.

agree trick on 0/1 bits: #agreeing bits = pm^T bits + (8 - pop(c)) with
pm = +-1 bit pattern of bucket c, so onehot = Relu(pm^T bits + 1-pop(c));
on +-1 signs onehot = Relu(pm^T s - 7).
Scan mask: m1 = onehot*rank is the 1-based rank at matches (0 elsewhere);
idx = m1 - 1 is the 0-based slot at matches and -1 (ignored) elsewhere.
Tables are fp16 (iota data j+1; integers <= 2048 are fp16-exact) because
the Pool engine cannot add int16.
"""

import numpy as np
import ml_dtypes

B, L, D, NH = 4, 2048, 64, 8
KMAX = 64
TABLE_ELEMS = 256   # > max bucket count (90 on this data); idx beyond -> never
HALF = L // 2
QTR = L // 4

_cache = {}


def _build_program():
    import concourse.bass as bass
    import concourse.mybir as mybir
    from concourse import bacc, tile
    from contextlib import ExitStack

    dt = mybir.dt
    Alu = mybir.AluOpType
    Act = mybir.ActivationFunctionType

    nc = bacc.Bacc("TRN2", target_bir_lowering=False, debug=False)

    # DRAM I/O (per-core shapes); kTgs/qTgs are host-prestacked [128, 512]:
    # rows 0:64 = chunk 2g (bf16 x^T), rows 64:128 = chunk 2g+1.
    kT0s_d = nc.declare_dram_parameter("kT0s", [128, QTR], dt.bfloat16, isOutput=False)
    kT1s_d = nc.declare_dram_parameter("kT1s", [128, QTR], dt.bfloat16, isOutput=False)
    qT0s_d = nc.declare_dram_parameter("qT0s", [128, QTR], dt.bfloat16, isOutput=False)
    qT1s_d = nc.declare_dram_parameter("qT1s", [128, QTR], dt.bfloat16, isOutput=False)
    # packed consts: cols 0:128 = pm (+-1 patterns at rows 0:8 and 64:72),
    # 128:200 = hi hash weights (block-diag [128,72]), 200:272 = lo,
    # 272 = 1-pop(c), 273 = -7, 274 = pop(c)-0.5.
    consts_d = nc.declare_dram_parameter("consts", [128, 276], dt.float16, isOutput=False)
    iota_d = nc.declare_dram_parameter("iotac", [128, L], dt.float16, isOutput=False)
    out_d = nc.declare_dram_parameter("out", [L, KMAX], dt.float16, isOutput=True)

    with ExitStack() as ctx:
        tc = ctx.enter_context(tile.TileContext(nc))
        sb = ctx.enter_context(tc.tile_pool(name="sb", bufs=1))
        hp = ctx.enter_context(tc.tile_pool(name="hp", bufs=3, space="PSUM"))
        ap = ctx.enter_context(tc.tile_pool(name="ap", bufs=3, space="PSUM"))
        gp = ctx.enter_context(tc.tile_pool(name="gp", bufs=1, space="PSUM"))

        # ---- loads: SP queue kT0s+consts+qT0s, Pool queue kT1s+qT1s,
        # ACT queue iota (dispatches only after the act-table preload).
        kT0s_sb = sb.tile([128, QTR], dt.bfloat16, tag="kT0s")
        nc.sync.dma_start(kT0s_sb[:], kT0s_d[:])
        kT1s_sb = sb.tile([128, QTR], dt.bfloat16, tag="kT1s")
        nc.gpsimd.dma_start(kT1s_sb[:], kT1s_d[:])
        consts_sb = sb.tile([128, 276], dt.float16, tag="consts")
        nc.sync.dma_start(consts_sb[:], consts_d[:])
        qT1s_sb = sb.tile([128, QTR], dt.bfloat16, tag="qT1s")
        nc.gpsimd.dma_start(qT1s_sb[:], qT1s_d[:])
        qT0s_sb = sb.tile([128, QTR], dt.bfloat16, tag="qT0s")
        nc.sync.dma_start(qT0s_sb[:], qT0s_d[:])
        iota_sb = sb.tile([128, L], dt.float16, tag="iota")
        nc.scalar.dma_start(iota_sb[:], iota_d[:])

        sgnc = consts_sb[:, 0:128]
        wpk_hi = consts_sb[:, 128:200]
        wpk_lo = consts_sb[:, 200:272]
        biasq = consts_sb[:, 272:273]
        bm7 = consts_sb[:, 273:274]

        # hash psum tiles; chunk 2g bits land at rows 0:8, chunk 2g+1 at
        # rows 64:72 (matmul SBUF operands need base partition 0/32/64).
        hpk1a = hp.tile([128, 256], dt.float32, tag="hp", name="hpk1a")
        hpk1b = hp.tile([128, 256], dt.float32, tag="hp", name="hpk1b")
        hpk2 = hp.tile([128, 512], dt.float32, tag="hp", name="hpk2")

        # PE warm-up: anchor the p-state clock (a >~3us idle resets the PE
        # ramp). Garbage results land in rows the real hash matmuls
        # overwrite with start=True.
        warm_sb = sb.tile([D, 64], dt.float16, tag="warm")
        nc.vector.memset(warm_sb[:], 0.0)
        for _ in range(2):
            nc.tensor.matmul(
                hpk1a[0:32, 0:64], lhsT=warm_sb[:, 0:32], rhs=warm_sb[:],
                start=True, stop=True,
            )

        def hash_pair(hpt, x2_ap):
            # x2 [128, n]: rows 0:64 = even chunk, 64:128 = odd chunk
            n = x2_ap.shape[-1]
            mm_hi = nc.tensor.matmul(
                hpt[0:72, 0:n], lhsT=wpk_hi, rhs=x2_ap, start=True, stop=False,
            )
            nc.tensor.matmul(
                hpt[0:72, 0:n], lhsT=wpk_lo, rhs=x2_ap, start=False, stop=True,
            )
            return mm_hi

        # ---- k side: bin (DVE, one op per stacked half) -> hash ----
        xk2 = [
            sb.tile([128, QTR], dt.float16, tag=f"xk2{g}", name=f"xk2{g}")
            for g in range(2)
        ]
        nc.vector.tensor_single_scalar(xk2[0][:], kT0s_sb[:], 0.0, Alu.is_gt)
        nc.vector.tensor_single_scalar(xk2[1][:], kT1s_sb[:], 0.0, Alu.is_gt)
        hash_pair(hpk1a, xk2[0][:, 0:256])
        hash_pair(hpk1b, xk2[0][:, 256:512])
        mm_hc = hash_pair(hpk2, xk2[1][:])

        # pair-1 bits as +-1 via ACT Sign in column-halves; pair-2 bits 0/1
        # via DVE is_gt (one-hot bias 2-2*popcount)
        s01k = sb.tile([128, 1024], dt.float16, tag="s01k")
        nc.scalar.activation(s01k[0:72, 0:256], hpk1a[0:72, :], Act.Sign)
        nc.scalar.activation(s01k[0:72, 256:512], hpk1b[0:72, :], Act.Sign)
        bitsC = nc.vector.tensor_single_scalar(
            s01k[0:72, 512:1024], hpk2[0:72, :], 0.0, Alu.is_gt
        )

        # ---- q side: bin (Pool, stacked) -> hash ----
        xq2 = [
            sb.tile([128, QTR], dt.float16, tag=f"xq2{g}", name=f"xq2{g}")
            for g in range(2)
        ]
        nc.gpsimd.tensor_single_scalar(xq2[1][:], qT1s_sb[:], 0.0, Alu.is_gt)
        nc.gpsimd.tensor_single_scalar(xq2[0][:], qT0s_sb[:], 0.0, Alu.is_gt)

        onehot = sb.tile([128, L], dt.float16, tag="onehot")
        sq = sb.tile([128, 1024], dt.float16, tag="sq")
        q1h = sb.tile([128, 1536], dt.float16, tag="q1h")
        hpq = [hp.tile([128, 512], dt.float32, tag="hp", name=f"hpq{g}") for g in range(2)]

        def agree(rhs_ap, n, name):
            t = ap.tile([128, n], dt.float32, tag="apt", name=name)
            r = rhs_ap.base_partition()
            mm = nc.tensor.matmul(
                t[:], lhsT=consts_sb[r : r + 8, 0:128],
                rhs=rhs_ap, start=True, stop=True,
            )
            return t, mm

        from concourse.tile_rust import add_dep_helper

        # PE emission: k agrees for early scan pieces first, q hashes behind
        apt_c0a, mm_c0a = agree(s01k[0:8, 0:256], 256, "apt_c0a")
        apt_c0b, mm_c0b = agree(s01k[0:8, 256:512], 256, "apt_c0b")
        apt_c1, mm_c1 = agree(s01k[64:72, 0:512], 512, "apt_c1")
        mm_hq1 = hash_pair(hpq[1], xq2[1][:])
        apt_c2, mm_c2 = agree(s01k[0:8, 512:1024], 512, "apt_c2")
        apt_c3, mm_c3 = agree(s01k[64:72, 512:1024], 512, "apt_c3")
        mm_hq0 = hash_pair(hpq[0], xq2[0][:])
        add_dep_helper(mm_hq1.ins, mm_c1.ins, sync=False,
                       reason="early k agrees before q hash on PE")
        add_dep_helper(mm_c2.ins, mm_c1.ins, sync=False,
                       reason="chunk-0/1 agrees first on PE")

        # k one-hot {0,1}: chunks 0,1 from +-1 signs (bias -7); 2,3 from
        # 0/1 bits (per-bucket bias 1-popcount)
        nc.scalar.activation(onehot[:, 0:256], apt_c0a[:], Act.Relu, bias=bm7)
        nc.scalar.activation(onehot[:, 256:512], apt_c0b[:], Act.Relu, bias=bm7)
        nc.scalar.activation(onehot[:, 512:1024], apt_c1[:], Act.Relu, bias=bm7)
        nc.scalar.activation(onehot[:, 1024:1536], apt_c2[:], Act.Relu, bias=biasq)
        nc.scalar.activation(onehot[:, 1536:2048], apt_c3[:], Act.Relu, bias=biasq)

        # q bits: both halves as +-1 ACT Signs after the k relu chain
        sq_e = nc.scalar.activation(sq[0:72, 512:1024], hpq[1][0:72, :], Act.Sign)
        sq_d = nc.scalar.activation(sq[0:72, 0:512], hpq[0][0:72, :], Act.Sign)

        # ---- rank keys within bucket: m1 = onehot*rank is the 1-based
        # rank at matches (0 elsewhere, Pool); idx = m1 - 1 is the 0-based
        # slot at matches, -1 (ignored) elsewhere (DVE, interleaved).
        # Last two pieces are 256 wide so the post-scan tail is short.
        rank = sb.tile([128, L], dt.float16, tag="rank")
        m1 = sb.tile([128, L], dt.float16, tag="m1")
        idx16 = sb.tile([128, L], dt.int16, tag="idx16")
        pieces = [(0, 256), (256, 512), (512, 1024), (1024, 1536),
                  (1536, 2048)]
        scan_inst = {}
        sub_inst = {}
        for i, (lo, hi) in enumerate(pieces):
            init = 0.0 if lo == 0 else rank[:, lo - 1 : lo]
            scan_inst[i] = nc.vector.tensor_tensor_scan(
                rank[:, lo:hi], onehot[:, lo:hi], onehot[:, lo:hi],
                init, Alu.add, Alu.bypass,
            )
            # the 256-wide tail pieces mask on DVE (no Pool round trip at
            # the end of the chain); earlier pieces mask on Pool
            mul_eng = nc.vector if i >= 4 else nc.gpsimd  # last piece DVE
            mul_eng.tensor_mul(m1[:, lo:hi], onehot[:, lo:hi], rank[:, lo:hi])
            sub_inst[i] = nc.vector.tensor_single_scalar(
                idx16[:, lo:hi], m1[:, lo:hi], 1.0, Alu.subtract
            )
        # keep the DVE scan cadence ahead of the subs
        for i in range(len(pieces) - 1):
            add_dep_helper(sub_inst[i].ins, scan_inst[i + 1].ins, sync=False,
                           reason="scan chain ahead of subs on DVE")
        # scatter ranges: quarters for pieces 0-3, eighths for the tail
        scat_ranges = [(0, 512), (512, 1024), (1024, 1536), (1536, 2048)]
        tabs = []
        for c, (lo, hi) in enumerate(scat_ranges):
            tab = sb.tile([128, TABLE_ELEMS], dt.float16, tag=f"table{c}")
            tabs.append(tab)
            nc.gpsimd.local_scatter(
                tab[:], iota_sb[:, lo:hi], idx16[:, lo:hi],
                channels=128, num_elems=TABLE_ELEMS, num_idxs=hi - lo,
            )

        aptq = {}
        for u in (3, 2, 0, 1):
            r = 64 * (u % 2)
            g = u // 2
            aptq[u], _ = agree(sq[r : r + 8, 512 * g : 512 * (g + 1)], 512, f"aptq{u}")
        # q one-hot {0,1}: c3 and c0 on ACT (Relu bias -7); c2 and c1 on
        # DVE (is_gt 7 on bf16 psum views), after the scan/sub chain.
        nc.scalar.activation(q1h[:, 1024:1536], aptq[3][:], Act.Relu, bias=bm7)
        nc.scalar.activation(q1h[:, 0:512], aptq[0][:], Act.Relu, bias=bm7)
        q1hx = sb.tile([128, 1024], dt.float16, tag="q1hx")
        q2_inst = nc.vector.tensor_single_scalar(
            q1hx[:], aptq[2][:].bitcast(dt.bfloat16), 7.0, Alu.is_gt
        )
        add_dep_helper(q2_inst.ins, sub_inst[4].ins, sync=False,
                       reason="k sub chain before c2 one-hot on DVE")
        q1hy = sb.tile([128, 1024], dt.float16, tag="q1hy")
        q1_inst = nc.vector.tensor_single_scalar(
            q1hy[:], aptq[1][:].bitcast(dt.bfloat16), 7.0, Alu.is_gt
        )
        add_dep_helper(q1_inst.ins, q2_inst.ins, sync=False,
                       reason="c2 before c1 one-hot on DVE")

        # merge tables on Pool (disjoint nonzero slots); columns 0..63 hold
        # the first 64 matches (j+1) per bucket
        m01 = sb.tile([128, KMAX], dt.float16, tag="m01")
        nc.gpsimd.tensor_add(m01[:], tabs[0][:, 0:KMAX], tabs[1][:, 0:KMAX])
        m23 = sb.tile([128, KMAX], dt.float16, tag="m23")
        nc.gpsimd.tensor_add(m23[:], tabs[2][:, 0:KMAX], tabs[3][:, 0:KMAX])
        tab16 = sb.tile([128, KMAX], dt.float16, tag="tab16")
        nc.gpsimd.tensor_add(tab16[:], m01[:], m23[:])

        # ---- gather per query, 4 waves of 4 matmuls (one per 512-query
        # chunk), in one-hot readiness order (c3, c2, c0, c1); each wave
        # copies its psum slice and DMAs immediately.
        # out[i, s] = sum_c q1h[c, i] * tab16[c, s]; psum partition p of
        # block t holds query 128t+p (host unpermutes).
        q1hx_v = q1hx[:].rearrange("c (i two) -> c i two", two=2)[:, :, 1]
        q1hy_v = q1hy[:].rearrange("c (i two) -> c i two", two=2)[:, :, 1]
        out_v = out_d[:].rearrange("(p t) s -> p (t s)", p=128)  # [128, 1024]
        opA = gp.tile([128, 512], dt.float32, tag="gatA", name="opA")
        opB = gp.tile([128, 512], dt.float32, tag="gatB", name="opB")
        psl = {0: opA[:, 0:256], 1: opA[:, 256:512], 2: opB[:, 0:256], 3: opB[:, 256:512]}

        def lhsT_for(t):
            u = t // 4
            if u == 2:
                return q1hx_v[:, 128 * (t % 4) : 128 * (t % 4 + 1)]
            if u == 1:
                return q1hy_v[:, 128 * (t % 4) : 128 * (t % 4 + 1)]
            base = {0: 0, 3: 1024}[u]
            return q1h[:, base + 128 * (t % 4) : base + 128 * (t % 4 + 1)]

        for u in (3, 2, 0, 1):
            for j in range(4):
                t = 4 * u + j
                nc.tensor.matmul(
                    psl[u][:, KMAX * j : KMAX * (j + 1)],
                    lhsT=lhsT_for(t), rhs=tab16[:],
                    start=True, stop=True,
                )
            osb = sb.tile([128, 256], dt.float16, tag=f"out{u}_sb", name=f"out{u}_sb")
            if u in (1, 2):
                nc.vector.tensor_copy(osb[:], psl[u][:])
            else:
                nc.scalar.activation(osb[:], psl[u][:], Act.Copy)
            if u in (3, 1):
                nc.sync.dma_start(out_v[:, 256 * u : 256 * (u + 1)], osb[:])
            else:
                nc.scalar.dma_start(out_v[:, 256 * u : 256 * (u + 1)], osb[:])

    nc.compile()
    return nc


def _get_nc():
    if "nc" not in _cache:
        _cache["nc"] = _build_program()
    return _cache["nc"]


def _make_in_maps(query, key, W):
    query = np.asarray(query, dtype=np.float32)
    key = np.asarray(key, dtype=np.float32)
    W = np.asarray(W, dtype=np.float32)

    def stack_half(xT, g):
        # [64, 2048] -> [128, 512]: rows 0:64 = chunk 2g, 64:128 = chunk 2g+1
        a = xT[:, 1024 * g : 1024 * g + 512]
        b = xT[:, 1024 * g + 512 : 1024 * (g + 1)]
        return np.ascontiguousarray(np.concatenate([a, b], axis=0))

    qs, ks = [], []
    for b in range(B):
        qT = query[b].T.astype(ml_dtypes.bfloat16)
        kT = key[b].T.astype(ml_dtypes.bfloat16)
        qs.append([stack_half(qT, 0), stack_half(qT, 1)])
        ks.append([stack_half(kT, 0), stack_half(kT, 1)])

    whi = W.astype(np.float16)
    wlo = (W - whi.astype(np.float32)).astype(np.float16)
    # compact block-diagonal pair weights [128, 144]: cols 0:8 map rows
    # 0:64 (even chunk) to psum rows 0:8; cols 64:72 map rows 64:128 to
    # psum rows 64:72.
    wpk = np.zeros((128, 144), np.float16)
    wpk[0:D, 0:NH] = whi
    wpk[D : 2 * D, D : D + NH] = whi
    wpk[0:D, 72 : 72 + NH] = wlo
    wpk[D : 2 * D, 72 + D : 72 + D + NH] = wlo

    iotac = np.ascontiguousarray(
        np.broadcast_to(
            (np.arange(L, dtype=np.float32) + 1.0).astype(np.float16)[None, :],
            (128, L),
        )
    )

    consts = []
    for h in range(2):
        cg = 128 * h + np.arange(128)  # global bucket ids of this half
        bits = ((cg[None, :] >> np.arange(NH)[:, None]) & 1).astype(np.float32)
        pm = (2.0 * bits - 1.0).astype(np.float16)  # [8, 128]
        arr = np.zeros((128, 276), np.float16)
        arr[0:NH, 0:128] = pm
        arr[D : D + NH, 0:128] = pm
        arr[:, 128:272] = wpk
        arr[:, 272] = (1.0 - bits.sum(axis=0)).astype(np.float16)
        arr[:, 273] = -7.0

        consts.append(arr)
    return [
        {
            "kT0s": ks[c // 2][0],
            "kT1s": ks[c // 2][1],
            "qT0s": qs[c // 2][0],
            "qT1s": qs[c // 2][1],
            "consts": consts[c % 2],
            "iotac": iotac,
        }
        for c in range(2 * B)
    ]


def _combine(results):
    # device layout: [128, 16*64], partition p col t*64+s <-> query 128t+p
    out = np.empty((B, L, KMAX), dtype=np.int64)
    for b in range(B):
        g = results[2 * b]["out"].astype(np.int64) + results[2 * b + 1]["out"].astype(
            np.int64
        )
        g = g.reshape(128, 16, KMAX).transpose(1, 0, 2).reshape(L, KMAX)
        out[b] = g - 1
    return out


def _run_spmd(in_maps, **kwargs):
    from concourse.bass_utils import run_bass_kernel_spmd

    return run_bass_kernel_spmd(_get_nc(), in_maps, list(range(2 * B)), **kwargs)


def kernel(query, key, W, head_idx=0, **_unused):
    in_maps = _make_in_maps(query, key, W)
    res = _run_spmd(in_maps)
    return _combine(res.results)
